# revision 9
# baseline (speedup 1.0000x reference)
"""Binarized complex-style dense layer on 8 TRN2 NeuronCores.

Computes out = sign(x + eps) @ K^T with K = [[br, -bi], [bi, br]],
br = sign(weight_real + eps), bi = sign(weight_imag + eps).

Sharding: data-parallel over the batch dim (131072 rows -> 16384 per core),
weights replicated. Forward only, so no collectives.

I/O precision: x and the weights are shipped to the device as bf16. This is
exact for this kernel: only sign(v + 1e-6) is consumed, and a bf16
round-to-nearest cast moves a value across the -1e-6 threshold only if it
lies within a relative 2^-9 of it (no element of this problem's inputs is
anywhere near that window). The outputs are sums of 256 terms of +-1, i.e.
even integers in [-256, 256], all exactly representable in bf16, so results
are stored as bf16 and upcast to f32 on the host with zero error. This
halves both DMA streams; every FLOP (binarize + matmul) stays on device.

Per-core pipeline (binarized values are +-1, exact in fp8e4; sums <= 256
are exact in fp32 PSUM):
  DMA x chunk (<=2048 rows, 8KB/partition contiguous descriptors) bf16->SBUF
  PE  transpose 128x128 bf16 sub-tiles -> PSUM bf16 (k on partitions)
  ACT sign(v + eps) PSUM bf16 -> SBUF fp8e4  (binarize fused into the copy;
      fp8 stationary tiles ride the 4x fast-weight-load path into the PE)
  PE  matmul xbT[k,b] @ kernelT[k,o] -> PSUM f32 [b, o]
  DVE copy PSUM f32 -> SBUF bf16
  DMA out chunk -> DRAM (GpSimd SWDGE ring, so stores never head-of-line
      block load issues on the Sync ring)
"""

import sys

import numpy as np

try:
    import concourse.bass  # noqa: F401
except ImportError:  # fresh env without the axon PYTHONPATH entries
    for p in ("/root/.axon_site/_ro/trn_rl_repo", "/opt/trn_rl_repo"):
        if p not in sys.path:
            sys.path.append(p)

N_CORES = 8
B_TOTAL = 131072
ROWS_PER_CORE = B_TOTAL // N_CORES  # 16384
FAN = 128
K2 = 2 * FAN  # 256 = 2*fan_in = 2*fan_out
EPS = 1e-6

_NC_CACHE = {}


def _build_nc(rows_per_core):
    from concourse import bacc, masks, mybir, tile

    f32 = mybir.dt.float32
    bf16 = mybir.dt.bfloat16
    fp8 = mybir.dt.float8e4
    Sign = mybir.ActivationFunctionType.Sign

    # Chunk schedule: 1MB mid-stream DMAs built from 8KB descriptors, small
    # chunks at both stream edges so the pipeline fills/drains quickly.
    if rows_per_core >= 16384:
        chunks = [256, 256, 512, 1024] + [2048] * 6 + [1024, 512, 256, 256]
    elif rows_per_core >= 1024:
        chunks = [1024] * (rows_per_core // 1024)
    else:
        chunks = [rows_per_core]
    assert sum(chunks) == rows_per_core
    assert all(c % 256 == 0 for c in chunks)

    nc = bacc.Bacc("TRN2", target_bir_lowering=False, debug=False)

    x_d = nc.dram_tensor("x", [rows_per_core, K2], bf16, kind="ExternalInput")
    wr_d = nc.dram_tensor("weight_real", [FAN, FAN], bf16, kind="ExternalInput")
    wi_d = nc.dram_tensor("weight_imag", [FAN, FAN], bf16, kind="ExternalInput")
    out_d = nc.dram_tensor("out", [rows_per_core, K2], bf16, kind="ExternalOutput")

    # DRAM views: a chunk is g groups of <=2048 rows; within group g_i,
    # partition p holds rows s + g_i*2048 + p*r + r_i, i.e. each partition
    # reads/writes g contiguous runs of r*512B (8KB max) per chunk. (g, r_i, k)
    # flattens to the same j*256 sub-tile offsets the compute loop uses.
    def chunk_view(t, start, rows):
        g = max(1, rows // 2048)
        r = rows // (128 * g)
        return t[start : start + rows, :].rearrange(
            "(g p r) k -> p g (r k)", g=g, p=128, r=r
        )

    with tile.TileContext(nc) as tc:
        with (
            tc.tile_pool(name="const", bufs=1) as const_pool,
            tc.tile_pool(name="kt", bufs=1) as kt_pool,
            tc.tile_pool(name="xin", bufs=6) as x_pool,
            tc.tile_pool(name="oout", bufs=5) as o_pool,
            tc.tile_pool(name="xbt", bufs=4) as xbt_pool,
            tc.tile_pool(name="ptp", bufs=4, space="PSUM") as tp_pool,
            tc.tile_pool(name="pout", bufs=2, space="PSUM") as po_pool,
        ):
            # First x chunk load goes out before anything else on the DMA
            # ring so the stream starts as early as possible.
            starts = [sum(chunks[:i]) for i in range(len(chunks))]
            x_tiles = {}
            xt0 = x_pool.tile([128, chunks[0] * 2], bf16, tag="xt")
            nc.sync.dma_start(out=xt0[:], in_=chunk_view(x_d, 0, chunks[0]))
            x_tiles[0] = xt0

            # Weight prep runs at priority 0 so kernelT is ready before the
            # first x sub-tiles come out of the transpose stage: the first
            # matmul must not wait on it.
            with tc.high_priority():
                # Identity first (it gates the very first PE transpose);
                # eps tiles ride the otherwise-idle DVE.
                ident = const_pool.tile([128, 128], bf16)
                masks.make_identity(nc, ident[:])
                eps_pos = const_pool.tile([128, 1], f32)
                nc.vector.memset(eps_pos[:], EPS)
                eps_neg = const_pool.tile([128, 1], f32)
                nc.vector.memset(eps_neg[:], -EPS)

                # Build kernelT [256 k, 256 o] as two [128, 256] fp8 tiles:
                #   kT0 = [ sign(wr^T) | sign(wi^T) ]   (k in [0,128))
                #   kT1 = [ -sign(wi^T) | sign(wr^T) ]  (k in [128,256))
                # Weight loads ride the Scalar HWDGE ring so the Sync ring
                # stays dedicated to the x stream.
                w_sb = const_pool.tile([128, 256], bf16)
                nc.scalar.dma_start(out=w_sb[:, 0:128], in_=wr_d[:])
                nc.scalar.dma_start(out=w_sb[:, 128:256], in_=wi_d[:])
                wt_ps = tp_pool.tile([128, 256], bf16, tag="tp")
                nc.tensor.transpose(wt_ps[:, 0:128], w_sb[:, 0:128], ident[:])
                nc.tensor.transpose(wt_ps[:, 128:256], w_sb[:, 128:256], ident[:])
                kt0 = kt_pool.tile([128, 256], fp8)
                kt1 = kt_pool.tile([128, 256], fp8)
                nc.scalar.activation(
                    kt0[:, 0:128], wt_ps[:, 0:128], Sign, bias=eps_pos[:]
                )
                nc.scalar.activation(
                    kt0[:, 128:256], wt_ps[:, 128:256], Sign, bias=eps_pos[:]
                )
                nc.scalar.activation(
                    kt1[:, 0:128], wt_ps[:, 128:256], Sign, bias=eps_neg[:], scale=-1.0
                )
                nc.scalar.activation(
                    kt1[:, 128:256], wt_ps[:, 0:128], Sign, bias=eps_pos[:]
                )

            for c, (start, rows) in enumerate(zip(starts, chunks)):
                n_j = rows // 128
                if c in x_tiles:
                    xt = x_tiles[c]
                else:
                    xt = x_pool.tile([128, rows * 2], bf16, tag="xt")
                    # Alternate loads between the two HWDGE rings so issue
                    # latency never serializes the stream (weights occupy
                    # the first two Scalar slots).
                    eng = nc.sync if c % 2 == 0 else nc.scalar
                    g = max(1, rows // 2048)
                    eng.dma_start(
                        out=xt[:].rearrange("p (g f) -> p g f", g=g),
                        in_=chunk_view(x_d, start, rows),
                    )
                ot = o_pool.tile([128, rows * 2], bf16, tag="ot")
                j0 = 0
                while j0 < n_j:
                    # Up to four 128-row sub-tiles share a 1-bank bf16 PSUM
                    # tile so the ACT fixed overhead amortizes over 1024 cols.
                    g = min(4, n_j - j0)
                    tp = tp_pool.tile([128, g * 256], bf16, tag="tp")
                    for h in range(g):
                        j = j0 + h
                        nc.tensor.transpose(
                            tp[:, h * 256 : h * 256 + 128],
                            xt[:, j * 256 : j * 256 + 128],
                            ident[:],
                        )
                        nc.tensor.transpose(
                            tp[:, h * 256 + 128 : h * 256 + 256],
                            xt[:, j * 256 + 128 : j * 256 + 256],
                            ident[:],
                        )
                    xbt = xbt_pool.tile([128, g * 256], fp8, tag="xbt")
                    nc.scalar.activation(xbt[:], tp[:], Sign, bias=eps_pos[:])
                    po = po_pool.tile([128, g * 256], f32, tag="po")
                    for h in range(g):
                        nc.tensor.matmul(
                            po[:, h * 256 : h * 256 + 256],
                            xbt[:, h * 256 : h * 256 + 128],
                            kt0[:],
                            start=True,
                            stop=False,
                        )
                        nc.tensor.matmul(
                            po[:, h * 256 : h * 256 + 256],
                            xbt[:, h * 256 + 128 : h * 256 + 256],
                            kt1[:],
                            start=False,
                            stop=True,
                        )
                    nc.vector.tensor_copy(
                        ot[:, j0 * 256 : (j0 + g) * 256], po[:]
                    )
                    j0 += g
                # Stores go out on the GpSimd (SWDGE) ring: a store waiting
                # on compute must not head-of-line block later load issues
                # on the Sync ring.
                nc.gpsimd.dma_start(
                    out=chunk_view(out_d, start, rows),
                    in_=ot[:].rearrange("p (g f) -> p g f", g=max(1, rows // 2048)),
                )

    nc.compile()
    return nc


def get_nc(rows_per_core=ROWS_PER_CORE):
    if rows_per_core not in _NC_CACHE:
        _NC_CACHE[rows_per_core] = _build_nc(rows_per_core)
    return _NC_CACHE[rows_per_core]


def kernel(x, weight_real, weight_imag, trace=False, tmpdir=None):
    import ml_dtypes
    from concourse import bass_utils

    bf16 = ml_dtypes.bfloat16
    # bf16 marshaling of x / weights is sign-exact here (see module docstring).
    x = np.asarray(x, dtype=np.float32).astype(bf16)
    wr = np.ascontiguousarray(np.asarray(weight_real, dtype=np.float32).astype(bf16))
    wi = np.ascontiguousarray(np.asarray(weight_imag, dtype=np.float32).astype(bf16))
    assert x.shape == (B_TOTAL, K2) and wr.shape == (FAN, FAN) and wi.shape == (FAN, FAN)

    nc = get_nc()
    in_maps = [
        {
            "x": x[i * ROWS_PER_CORE : (i + 1) * ROWS_PER_CORE],
            "weight_real": wr,
            "weight_imag": wi,
        }
        for i in range(N_CORES)
    ]
    res = bass_utils.run_bass_kernel_spmd(
        nc, in_maps, core_ids=list(range(N_CORES)), trace=trace, tmpdir=tmpdir
    )
    out = np.concatenate(
        [res.results[i]["out"].astype(np.float32) for i in range(N_CORES)], axis=0
    )
    if trace:
        return out, res
    return out


# revision 10
# speedup vs baseline: 1.1256x; 1.1256x over previous
"""Binarized complex-style dense layer on 8 TRN2 NeuronCores.

Computes out = sign(x + eps) @ K^T with K = [[br, -bi], [bi, br]],
br = sign(weight_real + eps), bi = sign(weight_imag + eps).

Sharding: data-parallel over the batch dim (131072 rows -> 16384 per core),
weights replicated. Forward only, so no collectives.

I/O precision: x and the weights are shipped to the device as bf16. This is
exact for this kernel: only sign(v + 1e-6) is consumed, and a bf16
round-to-nearest cast moves a value across the -1e-6 threshold only if it
lies within a relative 2^-9 of it (no element of this problem's inputs is
anywhere near that window). The outputs are sums of 256 terms of +-1, i.e.
even integers in [-256, 256], all exactly representable in bf16, so results
are stored as bf16 and upcast to f32 on the host with zero error. This
halves both DMA streams; every FLOP (binarize + matmul) stays on device.

Per-core pipeline (binarized values are +-1, exact in fp8e4; sums <= 256
are exact in fp32 PSUM):
  DMA x chunk (<=2048 rows, 8KB/partition contiguous descriptors) bf16->SBUF
  PE  transpose 128x128 bf16 sub-tiles -> PSUM bf16 (k on partitions)
  ACT sign(v + eps) PSUM bf16 -> SBUF fp8e4  (binarize fused into the copy;
      fp8 stationary tiles ride the 4x fast-weight-load path into the PE)
  PE  matmul xbT[k,b] @ kernelT[k,o] -> PSUM f32 [b, o]
  DVE copy PSUM f32 -> SBUF bf16
  DMA out chunk -> DRAM (GpSimd SWDGE ring, so stores never head-of-line
      block load issues on the Sync ring)
"""

import sys

import numpy as np

try:
    import concourse.bass  # noqa: F401
except ImportError:  # fresh env without the axon PYTHONPATH entries
    for p in ("/root/.axon_site/_ro/trn_rl_repo", "/opt/trn_rl_repo"):
        if p not in sys.path:
            sys.path.append(p)

N_CORES = 8
B_TOTAL = 131072
ROWS_PER_CORE = B_TOTAL // N_CORES  # 16384
FAN = 128
K2 = 2 * FAN  # 256 = 2*fan_in = 2*fan_out
EPS = 1e-6

_NC_CACHE = {}


def _build_nc(rows_per_core):
    from concourse import bacc, masks, mybir, tile

    f32 = mybir.dt.float32
    bf16 = mybir.dt.bfloat16
    fp8 = mybir.dt.float8e4
    Sign = mybir.ActivationFunctionType.Sign

    # Chunk schedule: 1MB mid-stream DMAs built from 8KB descriptors, small
    # chunks at both stream edges so the pipeline fills/drains quickly.
    if rows_per_core >= 16384:
        chunks = [256, 256, 512, 1024] + [2048] * 6 + [1024, 512, 256, 256]
    elif rows_per_core >= 1024:
        chunks = [1024] * (rows_per_core // 1024)
    else:
        chunks = [rows_per_core]
    assert sum(chunks) == rows_per_core
    assert all(c % 256 == 0 for c in chunks)

    nc = bacc.Bacc("TRN2", target_bir_lowering=False, debug=False)

    x_d = nc.dram_tensor("x", [rows_per_core, K2], bf16, kind="ExternalInput")
    wr_d = nc.dram_tensor("weight_real", [FAN, FAN], bf16, kind="ExternalInput")
    wi_d = nc.dram_tensor("weight_imag", [FAN, FAN], bf16, kind="ExternalInput")
    out_d = nc.dram_tensor("out", [rows_per_core, K2], bf16, kind="ExternalOutput")

    # DRAM views: a chunk is g groups of <=2048 rows; within group g_i,
    # partition p holds rows s + g_i*2048 + p*r + r_i, i.e. each partition
    # reads/writes g contiguous runs of r*512B (8KB max) per chunk. (g, r_i, k)
    # flattens to the same j*256 sub-tile offsets the compute loop uses.
    def chunk_view(t, start, rows):
        g = max(1, rows // 2048)
        r = rows // (128 * g)
        return t[start : start + rows, :].rearrange(
            "(g p r) k -> p g (r k)", g=g, p=128, r=r
        )

    with tile.TileContext(nc) as tc:
        with (
            tc.tile_pool(name="const", bufs=1) as const_pool,
            tc.tile_pool(name="kt", bufs=1) as kt_pool,
            tc.tile_pool(name="xin", bufs=6) as x_pool,
            tc.tile_pool(name="oout", bufs=5) as o_pool,
            tc.tile_pool(name="xbt", bufs=4) as xbt_pool,
            tc.tile_pool(name="ptp", bufs=4, space="PSUM") as tp_pool,
            tc.tile_pool(name="pout", bufs=2, space="PSUM") as po_pool,
        ):
            # First x chunk load goes out before anything else on the DMA
            # ring so the stream starts as early as possible.
            starts = [sum(chunks[:i]) for i in range(len(chunks))]
            x_tiles = {}
            xt0 = x_pool.tile([128, chunks[0] * 2], bf16, tag="xt")
            nc.sync.dma_start(out=xt0[:], in_=chunk_view(x_d, 0, chunks[0]))
            x_tiles[0] = xt0

            # Weight prep runs at priority 0 so kernelT is ready before the
            # first x sub-tiles come out of the transpose stage: the first
            # matmul must not wait on it.
            with tc.high_priority():
                # Identity first (it gates the very first PE transpose);
                # eps tiles ride the otherwise-idle DVE.
                ident = const_pool.tile([128, 128], bf16)
                masks.make_identity(nc, ident[:])
                eps_pos = const_pool.tile([128, 1], f32)
                nc.vector.memset(eps_pos[:], EPS)
                eps_neg = const_pool.tile([128, 1], f32)
                nc.vector.memset(eps_neg[:], -EPS)

                # Build kernelT [256 k, 256 o] as two [128, 256] fp8 tiles:
                #   kT0 = [ sign(wr^T) | sign(wi^T) ]   (k in [0,128))
                #   kT1 = [ -sign(wi^T) | sign(wr^T) ]  (k in [128,256))
                # Weight loads ride the Scalar HWDGE ring so the Sync ring
                # stays dedicated to the x stream.
                w_sb = const_pool.tile([128, 256], bf16)
                nc.scalar.dma_start(out=w_sb[:, 0:128], in_=wr_d[:])
                nc.scalar.dma_start(out=w_sb[:, 128:256], in_=wi_d[:])
                wt_ps = tp_pool.tile([128, 256], bf16, tag="tp")
                nc.tensor.transpose(wt_ps[:, 0:128], w_sb[:, 0:128], ident[:])
                nc.tensor.transpose(wt_ps[:, 128:256], w_sb[:, 128:256], ident[:])
                kt0 = kt_pool.tile([128, 256], fp8)
                kt1 = kt_pool.tile([128, 256], fp8)
                nc.scalar.activation(
                    kt0[:, 0:128], wt_ps[:, 0:128], Sign, bias=eps_pos[:]
                )
                nc.scalar.activation(
                    kt0[:, 128:256], wt_ps[:, 128:256], Sign, bias=eps_pos[:]
                )
                nc.scalar.activation(
                    kt1[:, 0:128], wt_ps[:, 128:256], Sign, bias=eps_neg[:], scale=-1.0
                )
                nc.scalar.activation(
                    kt1[:, 128:256], wt_ps[:, 0:128], Sign, bias=eps_pos[:]
                )

            for c, (start, rows) in enumerate(zip(starts, chunks)):
                n_j = rows // 128
                if c in x_tiles:
                    xt = x_tiles[c]
                else:
                    xt = x_pool.tile([128, rows * 2], bf16, tag="xt")
                    # Loads stay on the Sync ring: a dma_start on nc.scalar
                    # queues behind the in-flight SIGN activations on the
                    # Scalar engine queue and starves the stream. Only the
                    # second taper chunk rides Scalar, while it is idle.
                    eng = nc.scalar if c == 1 else nc.sync
                    g = max(1, rows // 2048)
                    eng.dma_start(
                        out=xt[:].rearrange("p (g f) -> p g f", g=g),
                        in_=chunk_view(x_d, start, rows),
                    )
                ot = o_pool.tile([128, rows * 2], bf16, tag="ot")
                j0 = 0
                while j0 < n_j:
                    # Up to four 128-row sub-tiles share a 1-bank bf16 PSUM
                    # tile so the ACT fixed overhead amortizes over 1024 cols.
                    g = min(4, n_j - j0)
                    tp = tp_pool.tile([128, g * 256], bf16, tag="tp")
                    for h in range(g):
                        j = j0 + h
                        nc.tensor.transpose(
                            tp[:, h * 256 : h * 256 + 128],
                            xt[:, j * 256 : j * 256 + 128],
                            ident[:],
                        )
                        nc.tensor.transpose(
                            tp[:, h * 256 + 128 : h * 256 + 256],
                            xt[:, j * 256 + 128 : j * 256 + 256],
                            ident[:],
                        )
                    xbt = xbt_pool.tile([128, g * 256], fp8, tag="xbt")
                    nc.scalar.activation(xbt[:], tp[:], Sign, bias=eps_pos[:])
                    po = po_pool.tile([128, g * 256], f32, tag="po")
                    for h in range(g):
                        nc.tensor.matmul(
                            po[:, h * 256 : h * 256 + 256],
                            xbt[:, h * 256 : h * 256 + 128],
                            kt0[:],
                            start=True,
                            stop=False,
                        )
                        nc.tensor.matmul(
                            po[:, h * 256 : h * 256 + 256],
                            xbt[:, h * 256 + 128 : h * 256 + 256],
                            kt1[:],
                            start=False,
                            stop=True,
                        )
                    nc.vector.tensor_copy(
                        ot[:, j0 * 256 : (j0 + g) * 256], po[:]
                    )
                    j0 += g
                # Stores go out on the GpSimd (SWDGE) ring: a store waiting
                # on compute must not head-of-line block later load issues
                # on the Sync ring.
                nc.gpsimd.dma_start(
                    out=chunk_view(out_d, start, rows),
                    in_=ot[:].rearrange("p (g f) -> p g f", g=max(1, rows // 2048)),
                )

    nc.compile()
    return nc


def get_nc(rows_per_core=ROWS_PER_CORE):
    if rows_per_core not in _NC_CACHE:
        _NC_CACHE[rows_per_core] = _build_nc(rows_per_core)
    return _NC_CACHE[rows_per_core]


def kernel(x, weight_real, weight_imag, trace=False, tmpdir=None):
    import ml_dtypes
    from concourse import bass_utils

    bf16 = ml_dtypes.bfloat16
    # bf16 marshaling of x / weights is sign-exact here (see module docstring).
    x = np.asarray(x, dtype=np.float32).astype(bf16)
    wr = np.ascontiguousarray(np.asarray(weight_real, dtype=np.float32).astype(bf16))
    wi = np.ascontiguousarray(np.asarray(weight_imag, dtype=np.float32).astype(bf16))
    assert x.shape == (B_TOTAL, K2) and wr.shape == (FAN, FAN) and wi.shape == (FAN, FAN)

    nc = get_nc()
    in_maps = [
        {
            "x": x[i * ROWS_PER_CORE : (i + 1) * ROWS_PER_CORE],
            "weight_real": wr,
            "weight_imag": wi,
        }
        for i in range(N_CORES)
    ]
    res = bass_utils.run_bass_kernel_spmd(
        nc, in_maps, core_ids=list(range(N_CORES)), trace=trace, tmpdir=tmpdir
    )
    out = np.concatenate(
        [res.results[i]["out"].astype(np.float32) for i in range(N_CORES)], axis=0
    )
    if trace:
        return out, res
    return out
